# revision 104
# baseline (speedup 1.0000x reference)
"""Trainium2 Bass kernel for EquivariantTPConv (gnn_message_passing).

Computation per edge e:
  sh  = SH_l012(edge_vec[e])                                  # [9]
  w   = (silu(edge_scalars[e] @ W1 + b1) @ W2 + b2)           # [3*64*16]
  x   = h_src[src_idx[e]]                                     # [64]
  feat[l,v] = sum_u x[u] * w[l,u,v] / 8                       # [3,16]
  msg = concat_l (feat[:,l,:,None] * sh_l[None,:])            # [144]
  out[d] = mean over {e: dst_idx[e]==d} msg[e]                # [n_dst,144]

Strategy (8 NeuronCores, edge/data parallel), "reversed TP" layout:
  - Host shards edges into 8 contiguous chunks, sorts each shard by dst,
    gathers h_src rows per shard, and builds per-tile one-hot "slot"
    matrices plus 1/count scales (sharding/data movement only).
  - W2 is host-permuted so MM2 runs TRANSPOSED: for each (l, q=v-pair)
    wT[(2u+v2), e] = sum_h W2[h,l,u,2q+v2] * h2[h,e]  (PE). b1 rides MM1 as
    a ones-row of esT so silu needs no bias operand.
  - The per-edge x-multiply happens in the (u,v2)-partition layout against
    xT2[(2u+v2), e] = x[e,u] (host-prepared row-duplicated transpose), split
    across engines to balance them: l0/l1 via Act full-tile casts
    (PSUM->SBUF bf16) + DVE 2x mult (l0) / Pool divide-by-reciprocal (l1:
    generic TensorTensor runs at 0.6 Q7 efficiency vs 0.42 for Multiply,
    using a host-prepared 1/x transpose); l2 via DVE mult straight from
    PSUM in two half-tiles (GPSIMD cannot read PSUM).
  - The u-reduction runs on PE with the per-edge product as lhsT:
    feat[e,16v] += prodT_lq^T @ R_q where R_q[(2u+v2), v] = [v == 2q+v2],
    accumulated over q in PSUM, with the b2 term folded in as one extra
    matmul (xT2^T @ b2rr) starting each accumulation group. This removes
    all vector-engine reduce work (output is only 16 columns per matmul).
  - msg = feat (x) sh: DVE evicts feat to SBUF bf16, Pool/DVE build the two
    outer-product blocks; one-hot matmul on PE does the within-tile
    segment-sum; 1/count is applied during eviction, alternating between
    Act and DVE per tile to balance queues.
  - PSUM (8 banks): 2x full wT [128,8,128]f32 (freed by Act casts), 2x half
    wT shared with the scatter ps tiles, h1, feat. Engine streams are
    emitted so ready work precedes cross-engine-dependent work, with the
    mid stage 3 tiles and tail stages 6-7 tiles behind the head.
  - Host unshard: segment-sum of the (sorted-label) slot rows across tiles
    and cores via np.add.reduceat == the cross-core all-reduce step.

  TimelineSim per-core: 133879 ns (baseline 210635 ns).
"""

import sys

for _p in ("/opt/trn_rl_repo", "/root/.axon_site/_ro/trn_rl_repo"):
    if _p not in sys.path:
        sys.path.append(_p)

import numpy as np

MUL_SRC = 64
MUL_DST = 16
N_PATHS = 3
SQ3 = 3.0 ** 0.5
SQ5 = 5.0 ** 0.5
SQ15 = 15.0 ** 0.5

N_CORES = 8
E_TOT = 50000
N_SRC = 10000
N_DST = 10000
ESD = 32
ESD1 = 33  # +1 ones-row carrying b1
HID = 128
WCOLS = N_PATHS * MUL_DST * MUL_SRC  # 3072 cols of W2T: (l, q, 2u+v2)
NLV = N_PATHS * MUL_DST  # 48
NQ = MUL_DST // 2  # 8 v-pairs per l
NQ2 = NQ // 2  # q's per wT half-tile (PSUM bank granularity)

T = 128  # edges per tile
BLK = 512  # edges per full MM1 block (last block is a 128-edge tail)
EC = E_TOT // N_CORES  # 6250 edges per core
NT = (EC + T - 1) // T  # 49 tiles
EP = NT * T  # 6272 padded edges per core
NB = (EP + BLK - 1) // BLK  # 13 blocks, last one covers a single tile

_PROG = None  # cached compiled program


def _build_program():
    from contextlib import ExitStack

    import concourse.tile as tile
    from concourse import bacc, mybir

    f32 = mybir.dt.float32
    bf16 = mybir.dt.bfloat16
    AF = mybir.ActivationFunctionType
    OP = mybir.AluOpType
    AX = mybir.AxisListType

    nc = bacc.Bacc(
        "TRN2",
        target_bir_lowering=False,
        debug=False,
        enable_asserts=False,
        num_devices=N_CORES,
    )

    # DRAM inputs; all big per-core tensors are partition-major on the host.
    esT_d = nc.dram_tensor("esT", [ESD1, EP], bf16, kind="ExternalInput")
    xT2_d = nc.dram_tensor("xT2", [HID, EP], bf16, kind="ExternalInput")
    xT2i_d = nc.dram_tensor("xT2i", [HID, EP], bf16, kind="ExternalInput")
    ev_d = nc.dram_tensor("ev", [T, NT, 3], f32, kind="ExternalInput")
    oh_d = nc.dram_tensor("oh", [T, NT, T], bf16, kind="ExternalInput")
    isl_d = nc.dram_tensor("isl", [T, NT], f32, kind="ExternalInput")
    W1_d = nc.dram_tensor("W1", [ESD1, HID], bf16, kind="ExternalInput")
    W2_d = nc.dram_tensor("W2T", [HID, WCOLS], bf16, kind="ExternalInput")
    b2_d = nc.dram_tensor("b2rr", [HID, NLV], bf16, kind="ExternalInput")
    R_d = nc.dram_tensor("Rq", [HID, NQ * MUL_DST], bf16, kind="ExternalInput")
    shc_d = nc.dram_tensor("shc", [T, 8], f32, kind="ExternalInput")
    out_d = nc.dram_tensor("outp", [T, NT, 144], f32, kind="ExternalOutput")

    with ExitStack() as ctx:
        tc = ctx.enter_context(tile.TileContext(nc))

        const = ctx.enter_context(tc.tile_pool(name="const", bufs=1))
        shp = ctx.enter_context(tc.tile_pool(name="shp", bufs=1))
        h2pool = ctx.enter_context(tc.tile_pool(name="h2pool", bufs=3))
        prodp = ctx.enter_context(tc.tile_pool(name="prodp", bufs=12))
        wbp = ctx.enter_context(tc.tile_pool(name="wbp", bufs=8))
        msgp = ctx.enter_context(tc.tile_pool(name="msgp", bufs=24))
        pwf = ctx.enter_context(tc.tile_pool(name="pwf", bufs=2, space="PSUM"))
        ph1 = ctx.enter_context(tc.tile_pool(name="ph1", bufs=1, space="PSUM"))
        pwp = ctx.enter_context(tc.tile_pool(name="pwp", bufs=2, space="PSUM"))
        pfeat = ctx.enter_context(tc.tile_pool(name="pfeat", bufs=1, space="PSUM"))

        # ---- resident loads; SP carries the edge streams (it has no compute),
        # GpSimd's queue carries the weights, ordered by first use ----
        W1s = const.tile([ESD1, HID], bf16)
        nc.gpsimd.dma_start(W1s[:], W1_d.ap())
        W2s = const.tile([HID, WCOLS], bf16)
        nc.gpsimd.dma_start(W2s[:, 0:1024], W2_d.ap()[:, 0:1024])
        nc.gpsimd.dma_start(W2s[:, 1024:], W2_d.ap()[:, 1024:])
        shcs = const.tile([T, 8], f32)
        nc.gpsimd.dma_start(shcs[:], shc_d.ap())
        b2s = const.tile([HID, NLV], bf16)
        nc.gpsimd.dma_start(b2s[:], b2_d.ap())
        Rs = const.tile([HID, NQ, MUL_DST], bf16)
        nc.gpsimd.dma_start(Rs[:], R_d.ap())

        ev_all = const.tile([T, NT, 3], f32)
        nc.sync.dma_start(ev_all[:, 0:8, :], ev_d.ap()[:, 0:8, :])
        nc.sync.dma_start(ev_all[:, 8:, :], ev_d.ap()[:, 8:, :])
        es_all = const.tile([ESD1, EP], bf16)
        nc.sync.dma_start(es_all[:, 0:BLK], esT_d.ap()[:, 0:BLK])
        nc.sync.dma_start(es_all[:, BLK : 2 * BLK], esT_d.ap()[:, BLK : 2 * BLK])
        xT2_all = const.tile([HID, EP], bf16)
        nc.sync.dma_start(xT2_all[:, 0 : 4 * T], xT2_d.ap()[:, 0 : 4 * T])
        isl_all = const.tile([T, NT], f32)
        nc.sync.dma_start(isl_all[:], isl_d.ap())
        oh_all = const.tile([T, NT, T], bf16)
        nc.sync.dma_start(oh_all[:, 0:4, :], oh_d.ap()[:, 0:4, :])
        xT2i_all = const.tile([HID, EP], bf16)
        nc.sync.dma_start(xT2i_all[:, 0 : 4 * T], xT2i_d.ap()[:, 0 : 4 * T])
        nc.sync.dma_start(es_all[:, 2 * BLK :], esT_d.ap()[:, 2 * BLK :])
        nc.sync.dma_start(xT2_all[:, 4 * T :], xT2_d.ap()[:, 4 * T :])
        nc.sync.dma_start(oh_all[:, 4:, :], oh_d.ap()[:, 4:, :])
        nc.sync.dma_start(xT2i_all[:, 4 * T :], xT2i_d.ap()[:, 4 * T :])
        negone = const.tile([T, 1], f32)
        nc.vector.memset(negone[:], -1.0)

        ob_all = const.tile([T, NT, 144], f32)

        # ---- SH prologue: split head/tail so the sqrt chain starts early ----
        sq_all = shp.tile([T, NT, 3], f32)
        r2_all = shp.tile([T, NT], f32)
        rn_all = shp.tile([T, NT], f32)
        for lo, hi in ((0, 8), (8, NT)):
            nc.vector.tensor_tensor(
                sq_all[:, lo:hi, :], ev_all[:, lo:hi, :], ev_all[:, lo:hi, :],
                op=OP.mult,
            )
            nc.vector.tensor_reduce(
                r2_all[:, lo:hi], sq_all[:, lo:hi, :], axis=AX.X, op=OP.add
            )
            nc.scalar.activation(rn_all[:, lo:hi], r2_all[:, lo:hi], AF.Sqrt)

        def bc(ap_, shape):
            return ap_.to_broadcast(shape)

        sh_all = shp.tile([T, NT, 9], f32)

        def emit_sh_part2():
            inv_all = shp.tile([T, NT], f32)
            nc.vector.reciprocal(inv_all[:], rn_all[:])
            inv2_all = shp.tile([T, NT], f32)
            nc.vector.tensor_tensor(inv2_all[:], inv_all[:], inv_all[:], op=OP.mult)
            i1 = inv_all[:].rearrange("p (t o) -> p t o", o=1)
            i2 = inv2_all[:].rearrange("p (t o) -> p t o", o=1)
            nc.vector.tensor_tensor(
                sh_all[:, :, 1:4], ev_all[:], bc(i1, [T, NT, 3]), op=OP.mult
            )
            pq_all = shp.tile([T, NT, 2], f32)
            nc.vector.tensor_tensor(
                pq_all[:], ev_all[:, :, 0:2], ev_all[:, :, 1:3], op=OP.mult
            )
            nc.vector.tensor_tensor(
                sh_all[:, :, 4:6], pq_all[:], bc(i2, [T, NT, 2]), op=OP.mult
            )
            t6_all = shp.tile([T, NT], f32)
            nc.vector.tensor_tensor(
                t6_all[:].rearrange("p (t o) -> p t o", o=1),
                sq_all[:, :, 2:3],
                i2,
                op=OP.mult,
            )
            nc.scalar.activation(
                sh_all[:, :, 6], t6_all[:], AF.Identity, bias=negone[:, 0:1], scale=3.0
            )
            xz_all = shp.tile([T, NT, 1], f32)
            nc.vector.tensor_tensor(
                xz_all[:], ev_all[:, :, 0:1], ev_all[:, :, 2:3], op=OP.mult
            )
            nc.vector.tensor_tensor(sh_all[:, :, 7:8], xz_all[:], i2, op=OP.mult)
            d2_all = shp.tile([T, NT, 1], f32)
            nc.vector.tensor_tensor(
                d2_all[:], sq_all[:, :, 0:1], sq_all[:, :, 1:2], op=OP.subtract
            )
            nc.vector.tensor_tensor(sh_all[:, :, 8:9], d2_all[:], i2, op=OP.mult)
            shc3 = shcs[:].rearrange("p (o c) -> p o c", o=1)
            nc.vector.tensor_tensor(
                sh_all[:, :, 1:9], sh_all[:, :, 1:9], bc(shc3, [T, NT, 8]), op=OP.mult
            )
            nc.vector.tensor_copy(shb_all[:], sh_all[:])

        # ---- main pipeline ----
        # Per iteration i (tile t = i), interleaved so each engine's in-order
        # stream lists ready work before work that waits on another engine:
        #   PE : MM1(+4) | MM2T-l0(t), MM2T-l1(t) | mid(t-2): b2 + 24 reduce
        #        matmuls | scatter(t-3) | MM2T-l2(t)
        #   Act: silu(+4) | cast-l0(t), cast-l1(t)
        #   DVE: fb(t-3), msg-l2(t-3), evict(t-3) | mult-l2(t) | mult-l0(t)
        #   Pool: msg-copy16(t-3), msg-l1(t-3) | mult-l1(t)
        shb_all = const.tile([T, NT, 9], bf16)
        state = {}  # t -> dict with prods/feat etc.

        h2_by_block = {}

        def emit_mm1_block(b):
            if b >= NB or b in h2_by_block:
                return
            nbt = min(4, NT - b * 4)
            bw = nbt * T
            h1 = ph1.tile([HID, bw], f32, tag="h1", name=f"h1_{b}")
            nc.tensor.matmul(
                h1[:], W1s[:], es_all[:, b * BLK : b * BLK + bw],
                start=True, stop=True,
            )
            h2 = h2pool.tile([HID, bw], bf16, tag="h2", name=f"h2_{b}")
            nc.scalar.activation(h2[:], h1[:], AF.Silu)
            h2_by_block[b] = h2

        def emit_mm1(t):
            b, tt = divmod(t, 4)
            if t == 0:
                emit_mm1_block(0)
                emit_mm1_block(1)
            elif tt == 2:
                emit_mm1_block(b + 1)

        def emit_mm2t(t, l, h):
            b, tt = divmod(t, 4)
            h2 = h2_by_block[t // 4]
            wT = pwp.tile([HID, NQ2, T], f32, tag="wT", name=f"wT{t}_{l}_{h}")
            for qq in range(NQ2):
                q = h * NQ2 + qq
                nc.tensor.matmul(
                    wT[:, qq, :],
                    W2s[:, (l * NQ + q) * T : (l * NQ + q + 1) * T],
                    h2[:, tt * T : (tt + 1) * T],
                    start=True,
                    stop=True,
                )
            return wT

        def emit_mm2t_full(t, l):
            b, tt = divmod(t, 4)
            h2 = h2_by_block[t // 4]
            wT = pwf.tile([HID, NQ, T], f32, tag="wTf", name=f"wTf{t}_{l}")
            for q in range(NQ):
                nc.tensor.matmul(
                    wT[:, q, :],
                    W2s[:, (l * NQ + q) * T : (l * NQ + q + 1) * T],
                    h2[:, tt * T : (tt + 1) * T],
                    start=True,
                    stop=True,
                )
            return wT

        def emit_cast_full(t, l, wT):
            wb = wbp.tile([HID, NQ, T], bf16, tag=f"wb{l}", name=f"wb{l}_{t}")
            nc.scalar.activation(wb[:], wT[:], AF.Copy)
            return wb

        def xsl(t):
            return xT2_all[:, t * T : (t + 1) * T].rearrange("p (o e) -> p o e", o=1)

        def emit_mult_dve2x(t, l, wb):
            prod = prodp.tile([HID, NQ, T], bf16, tag=f"prod{l}", name=f"prod{l}_{t}")
            nc.vector.tensor_tensor(
                prod[:], wb[:], bc(xsl(t), [HID, NQ, T]), op=OP.mult
            )
            return prod

        def emit_mult_pool(t, l, wb):
            prod = prodp.tile([HID, NQ, T], bf16, tag=f"prod{l}", name=f"prod{l}_{t}")
            # last tiles: DVE 2x finishes the final dependency chain sooner
            eng = nc.vector if t >= NT - 2 else nc.gpsimd
            eng.tensor_tensor(
                prod[:], wb[:], bc(xsl(t), [HID, NQ, T]), op=OP.mult
            )
            return prod

        def emit_mult_dve_psum_half(t, l, h, wT, prod):
            nc.vector.tensor_tensor(
                prod[:, h * NQ2 : (h + 1) * NQ2, :],
                wT[:],
                bc(xsl(t), [HID, NQ2, T]),
                op=OP.mult,
            )

        def emit_mid(t):
            prods = state[t]["prods"]
            feat = pfeat.tile([T, NLV], f32, tag="feat", name=f"feat{t}")
            for l in range(3):
                nc.tensor.matmul(
                    feat[:, l * MUL_DST : (l + 1) * MUL_DST],
                    xT2_all[:, t * T : (t + 1) * T],
                    b2s[:, l * MUL_DST : (l + 1) * MUL_DST],
                    start=True,
                    stop=False,
                )
                for q in range(NQ):
                    nc.tensor.matmul(
                        feat[:, l * MUL_DST : (l + 1) * MUL_DST],
                        prods[l][:, q, :],
                        Rs[:, q, :],
                        start=False,
                        stop=(q == NQ - 1),
                    )
            state[t]["feat"] = feat

        def emit_tail_pre(t):
            # DVE: feat PSUM -> SBUF bf16; then msg parts
            feat = state[t]["feat"]
            fb = msgp.tile([T, NLV], bf16, tag="fb", name=f"fb{t}")
            nc.vector.tensor_copy(fb[:], feat[:])
            msg = msgp.tile([T, 144], bf16, tag="msg", name=f"msg{t}")
            nc.gpsimd.tensor_copy(msg[:, 0:16], fb[:, 0:16])
            nc.gpsimd.tensor_tensor(
                msg[:, 16:64].rearrange("p (v m) -> p v m", m=3),
                fb[:, 16:32]
                .rearrange("p (v o) -> p v o", o=1)
                .to_broadcast([T, 16, 3]),
                shb_all[:, t, 1:4]
                .rearrange("p (o m) -> p o m", o=1)
                .to_broadcast([T, 16, 3]),
                op=OP.mult,
            )
            nc.vector.tensor_tensor(
                msg[:, 64:144].rearrange("p (v m) -> p v m", m=5),
                fb[:, 32:48]
                .rearrange("p (v o) -> p v o", o=1)
                .to_broadcast([T, 16, 5]),
                shb_all[:, t, 4:9]
                .rearrange("p (o m) -> p o m", o=1)
                .to_broadcast([T, 16, 5]),
                op=OP.mult,
            )
            state[t]["msg"] = msg

        ob_bounds = list(range(0, NT, 4)) + [NT]

        def emit_tail_post(t):
            msg = state[t]["msg"]
            ps = pwp.tile([T, 144], f32, tag="wT", name=f"ps{t}")
            nc.tensor.matmul(ps[:], oh_all[:, t, :], msg[:], start=True, stop=True)
            if t % 2 == 0:
                nc.scalar.activation(
                    ob_all[:, t, :], ps[:], AF.Copy, scale=isl_all[:, t : t + 1]
                )
            else:
                nc.vector.tensor_scalar(
                    ob_all[:, t, :], ps[:], isl_all[:, t : t + 1], None, op0=OP.mult
                )
            del state[t]
            if t + 1 in ob_bounds:
                c = ob_bounds.index(t + 1) - 1
                lo, hi = ob_bounds[c], ob_bounds[c + 1]
                nc.sync.dma_start(
                    out_d.ap()[:, lo:hi, :], ob_all[:, lo:hi, :]
                )

        OFF_MID = 3
        OFF_PRE = 6
        OFF_POST = 7
        for i in range(NT + OFF_POST):
            t = i if i < NT else None
            tm = i - OFF_MID if OFF_MID <= i < NT + OFF_MID else None
            tp = i - OFF_PRE if OFF_PRE <= i < NT + OFF_PRE else None
            tq = i - OFF_POST if i >= OFF_POST else None

            # scatter+evict of the oldest tile first: its msg is long ready
            if tq is not None:
                emit_tail_post(tq)
            if t is not None:
                emit_mm1(t)
                wT0 = emit_mm2t_full(t, 0)
                wb0 = emit_cast_full(t, 0, wT0)
                wT1 = emit_mm2t_full(t, 1)
                wb1 = emit_cast_full(t, 1, wT1)
                state[t] = {}
            if tm is not None:
                emit_mid(tm)
            if tp is not None:
                emit_tail_pre(tp)
            if t is not None:
                prod2 = prodp.tile(
                    [HID, NQ, T], bf16, tag="prod2", name=f"prod2_{t}"
                )
                if t >= NT - 2:
                    for h in range(2):
                        wT = emit_mm2t(t, 2, h)
                        nc.scalar.activation(
                            prod2[:, h * NQ2 : (h + 1) * NQ2, :], wT[:], AF.Copy
                        )
                    # overwrite with the actual product on DVE (2x from SBUF)
                    nc.vector.tensor_tensor(
                        prod2[:], prod2[:], bc(xsl(t), [HID, NQ, T]), op=OP.mult
                    )
                else:
                    for h in range(2):
                        wT = emit_mm2t(t, 2, h)
                        emit_mult_dve_psum_half(t, 2, h, wT, prod2)
                prod1 = emit_mult_pool(t, 1, wb1)
                prod0 = emit_mult_dve2x(t, 0, wb0)
                state[t]["prods"] = (prod0, prod1, prod2)
            if t == 1:
                emit_sh_part2()

    nc.compile()
    return nc


def _get_program():
    global _PROG
    if _PROG is None:
        _PROG = _build_program()
    return _PROG


def _prep_core(c, h_src, edge_vec, edge_scalars, src_idx, dst_idx, inv_cnt):
    """Shard + sort + gather + one-hot build for one core (partition-major)."""
    import ml_dtypes

    bf = ml_dtypes.bfloat16
    lo, hi = c * EC, (c + 1) * EC
    d = dst_idx[lo:hi]
    order = np.argsort(d, kind="stable")
    d_s = d[order]
    s_s = src_idx[lo:hi][order]

    esT = np.zeros((ESD1, EP), np.float32)
    esT[:ESD, :EC] = edge_scalars[lo:hi][order].T
    esT[ESD, :] = 1.0
    x = np.zeros((EP, MUL_SRC), np.float32)
    x[:EC] = h_src[s_s]
    ev = np.zeros((EP, 3), np.float32)
    ev[:EC] = edge_vec[lo:hi][order]
    ev[EC:, 0] = 1.0

    d_pad = np.full(EP, N_DST, np.int64)
    d_pad[:EC] = d_s

    oh = np.zeros((EP, T), np.float32)
    isl = np.ones((EP,), np.float32)
    labels = np.full(NT * T, N_DST, np.int64)
    dt2 = d_pad.reshape(NT, T)
    for t in range(NT):
        uniq, inv = np.unique(dt2[t], return_inverse=True)
        oh[t * T : (t + 1) * T, :][np.arange(T), inv] = 1.0
        labels[t * T : t * T + len(uniq)] = uniq
        real = uniq[uniq < N_DST]
        isl[t * T : t * T + len(real)] = inv_cnt[real]

    # xT2: row (2u+v2) = x[:, u] for v2 in {0,1}
    xT2 = np.repeat(x.T, 2, axis=0)  # [128, EP]
    with np.errstate(divide="ignore"):
        xT2i = np.float32(1.0) / xT2.astype(bf).astype(np.float32)

    # partition-major device layouts: [p, t, ...] = row t*T + p
    def pmaj(a):
        return np.ascontiguousarray(a.reshape(NT, T, -1).transpose(1, 0, 2))

    return (
        {
            "esT": esT.astype(bf),
            "xT2": np.ascontiguousarray(xT2).astype(bf),
            "xT2i": np.ascontiguousarray(xT2i).astype(bf),
            "ev": pmaj(ev),
            "oh": pmaj(oh).astype(bf),
            "isl": np.ascontiguousarray(isl.reshape(NT, T).T),
        },
        labels,
    )


def kernel(**inputs):
    import ml_dtypes

    from concourse import bass_utils

    bf = ml_dtypes.bfloat16

    h_src = np.asarray(inputs["h_src"], np.float32)
    edge_vec = np.asarray(inputs["edge_vec"], np.float32)
    edge_scalars = np.asarray(inputs["edge_scalars"], np.float32)
    W1 = np.asarray(inputs["W1"], np.float32)
    b1 = np.asarray(inputs["b1"], np.float32)
    W2 = np.asarray(inputs["W2"], np.float32)
    b2 = np.asarray(inputs["b2"], np.float32)
    src_idx = np.asarray(inputs["src_idx"]).astype(np.int64)
    dst_idx = np.asarray(inputs["dst_idx"]).astype(np.int64)
    n_dst = int(inputs["n_dst"])
    assert n_dst == N_DST

    nc = _get_program()

    cnt = np.bincount(dst_idx, minlength=N_DST)
    inv_cnt = (1.0 / np.maximum(cnt, 1)).astype(np.float32)

    scale = 1.0 / np.sqrt(MUL_SRC)
    # W2T[h, l, q, 2u+v2] = W2[h, l, u, 2q+v2] * scale
    W2r = W2.reshape(HID, N_PATHS, MUL_SRC, NQ, 2)  # [h, l, u, q, v2]
    W2T = (W2r.transpose(0, 1, 3, 2, 4) * scale).reshape(HID, WCOLS)
    # b2rr[2u+v2, (l,v)] = b2[l, u, v] * scale / 2
    b2r = (b2.reshape(N_PATHS, MUL_SRC, MUL_DST).transpose(1, 0, 2) * scale).reshape(
        MUL_SRC, NLV
    )
    b2rr = np.repeat(b2r * 0.5, 2, axis=0)  # [128, 48]
    # Rq[2u+v2, q, v] = [v == 2q+v2]
    Rq = np.zeros((HID, NQ, MUL_DST), np.float32)
    p = np.arange(HID)
    for q in range(NQ):
        Rq[p, q, 2 * q + (p & 1)] = 1.0

    shc = np.broadcast_to(
        np.array(
            [SQ3, SQ3, SQ3, SQ15, SQ15, 0.5 * SQ5, SQ15, 0.5 * SQ15], np.float32
        ),
        (T, 8),
    ).copy()

    W1a = np.concatenate([W1, b1.reshape(1, HID)], axis=0)
    shared = {
        "W1": np.ascontiguousarray(W1a).astype(bf),
        "W2T": np.ascontiguousarray(W2T).astype(bf),
        "b2rr": np.ascontiguousarray(b2rr).astype(bf),
        "Rq": Rq.reshape(HID, NQ * MUL_DST).astype(bf),
        "shc": shc,
    }

    in_maps = []
    labels_all = []
    for c in range(N_CORES):
        m, labels = _prep_core(
            c, h_src, edge_vec, edge_scalars, src_idx, dst_idx, inv_cnt
        )
        m.update(shared)
        in_maps.append(m)
        labels_all.append(labels)

    import time

    t0 = time.perf_counter()
    res = bass_utils.run_bass_kernel_spmd(nc, in_maps, core_ids=list(range(N_CORES)))
    t1 = time.perf_counter()
    kernel.last_device_wall_s = t1 - t0

    # outp is [T, NT, 144] partition-major; row (t, p) lives at [p, t, :]
    rows = np.concatenate(
        [
            res.results[c]["outp"].transpose(1, 0, 2).reshape(NT * T, 144)
            for c in range(N_CORES)
        ],
        axis=0,
    )
    labels = np.concatenate(labels_all)

    order = np.argsort(labels, kind="stable")
    lab_s = labels[order]
    rows_s = rows[order]
    starts = np.concatenate(([0], np.flatnonzero(np.diff(lab_s)) + 1))
    sums = np.add.reduceat(rows_s, starts, axis=0)
    out = np.zeros((N_DST + 1, 144), np.float32)
    out[lab_s[starts]] = sums
    return out[:N_DST]
